# revision 14
# baseline (speedup 1.0000x reference)
"""Trainium2 Bass kernel for batched weighted scatter-add (AttentionCopy).

Computes out[b, o, v] = sum_i attn[b, o, i] * (ids[b, i] == v)
for ids [16, 512] int32 in [0, 50000), attn [16, 32, 512] f32,
out [16, 32, 50000] f32.

Strategy: pure data parallel over the batch dim — 2 batches per core on 8
cores. Per batch the output is built *densely* on-device via a two-level
one-hot factorization of the vocab index (v = g*2000 + lo, 25 groups):

  out[o, g, lo] = sum_i (attnT[i, o] * (hi_i == g)) * (lo_i == lo)
                = (G_(o,g)).T @ Alo      (PE matmul, contraction over i)

The masked matrix G.T[i, (o, g)] and the one-hot Alo[i, lo] are built with
iota-constant compares on the vector engine. With o-major (o, g) pairs and
V2 = 1250 (40 * 1250 = 50000 exactly), each [128, 1250] f32 PSUM tile maps
to a *flat contiguous* 640 KB run of the row-major [32, 50000] output, so
every DRAM write is a full-partition coalesced DMA (all 16 SDMA engines),
and the 1280 pairs split into exactly 10 full 128-partition tiles.

hi = ids // 1250 uses the round-to-nearest int cast of
(ids + 0.5) * (1/1250) - 0.5, validated exhaustively on HW for [0, 50000).
"""

import sys

sys.path.insert(0, "/opt/trn_rl_repo")

import numpy as np

NCORES = 8
B, O, I = 16, 32, 512
SIZE = 50000
BPC = B // NCORES  # batches per core
V2 = 1250  # lo range (3 PSUM banks per tile)
V1 = 40  # number of vocab groups: 40 * 1250 = 50000 exactly
PAIRS = O * V1  # 1280 (o, g) pairs per batch, o-major
NTILES = PAIRS // 128  # exactly 10 full 128-pair tiles
NCHUNK = I // 128  # 4 contraction chunks
# matmul N-slices of V2, each within one 2 KiB PSUM bank
NSLICES = [(0, 512), (512, 1024), (1024, 1250)]

_cache = {}


def _build(mm_dtype="bfloat16"):
    import concourse.bacc as bacc
    import concourse.mybir as mybir
    import concourse.tile as tile

    f32 = mybir.dt.float32
    f16 = mybir.dt.float16
    mmdt = getattr(mybir.dt, mm_dtype)
    i32 = mybir.dt.int32
    Alu = mybir.AluOpType

    nc = bacc.Bacc("TRN2", target_bir_lowering=False, debug=False, num_devices=NCORES)

    # ids pre-tiled on host to [BPC, 128, NCHUNK] with [p, c] = ids[c*128 + p]
    ids_d = nc.dram_tensor("ids", [128, BPC * NCHUNK], i32, kind="ExternalInput").ap()
    attn_d = nc.dram_tensor("attn", [BPC, I, O], f32, kind="ExternalInput").ap()
    gidx_d = nc.dram_tensor("gidx", [128, PAIRS], f16, kind="ExternalInput").ap()
    lov_d = nc.dram_tensor("lov", [128, V2], f16, kind="ExternalInput").ap()
    out_d = nc.dram_tensor("out", [BPC, O, SIZE], f32, kind="ExternalOutput").ap()

    with tile.TileContext(nc) as tc:
        with (
            tc.tile_pool(name="const", bufs=1) as constp,
            tc.tile_pool(name="inp", bufs=2) as inp,
            tc.tile_pool(name="idx", bufs=2) as idxp,
            tc.tile_pool(name="gt", bufs=2) as gtp,
            tc.tile_pool(name="outs", bufs=4) as outp,
            tc.tile_pool(name="psmm", bufs=2, space="PSUM") as psmm,
        ):
            warm = constp.tile([128, 256], mmdt)
            nc.gpsimd.memset(warm[:], 0)
            wps = psmm.tile([128, 256], f32, tag="mm")
            for _ in range(52):
                nc.tensor.matmul(out=wps[:, :256], lhsT=warm[:, :128],
                                 rhs=warm[:, :256], start=True, stop=True)
            NBC = BPC * NCHUNK
            ids_all = idxp.tile([128, NBC], i32)
            nc.scalar.dma_start(out=ids_all[:], in_=ids_d[:])
            at_alls = []
            for b in range(BPC):
                at_alls.append(inp.tile([128, NCHUNK * O], f32, tag=f"attn{b}", name=f"at_all{b}"))
            lov = constp.tile([128, V2], f16)
            nc.sync.dma_start(out=lov[:], in_=lov_d[:])
            gidx = constp.tile([128, PAIRS], f16)
            nc.sync.dma_start(out=gidx[:], in_=gidx_d[:])
            for b in range(BPC):
                nc.sync.dma_start(
                    out=at_alls[b][:].rearrange("p (c o) -> p c o", o=O),
                    in_=attn_d[b].rearrange("(c p) o -> p c o", p=128),
                )

            # hi = ids // 1250 via RTN int cast of (ids+0.5)/1250 - 0.5
            # (exact for [0, 50000), verified on HW); lo = ids - 1250*hi
            # computed for both batches in one [128, 8] pass
            ids_f = idxp.tile([128, NBC], f32)
            nc.vector.tensor_copy(out=ids_f[:], in_=ids_all[:])
            tq = idxp.tile([128, NBC], f32)
            nc.vector.tensor_scalar(out=tq[:], in0=ids_f[:], scalar1=0.5,
                                    scalar2=float(np.float32(1.0 / V2)),
                                    op0=Alu.add, op1=Alu.mult)
            hi_i = idxp.tile([128, NBC], i32)
            nc.vector.tensor_scalar(out=hi_i[:], in0=tq[:], scalar1=0.5,
                                    scalar2=None, op0=Alu.subtract)
            hi_fa = idxp.tile([128, NBC], f32)
            nc.vector.tensor_copy(out=hi_fa[:], in_=hi_i[:])
            lo_fa = idxp.tile([128, NBC], f32)
            nc.vector.scalar_tensor_tensor(out=lo_fa[:], in0=hi_fa[:],
                                           scalar=float(-V2), in1=ids_f[:],
                                           op0=Alu.mult, op1=Alu.add)

            for b in range(BPC):
                at_all = at_alls[b]
                hi_f = hi_fa[:, b * NCHUNK : (b + 1) * NCHUNK]
                lo_f = lo_fa[:, b * NCHUNK : (b + 1) * NCHUNK]

                gt = gtp.tile([128, NCHUNK * PAIRS], mmdt, tag="gt")
                alo = gtp.tile([128, NCHUNK * V2], mmdt, tag="alo")
                bhs = []
                for c in range(NCHUNK):
                    nc.vector.tensor_scalar(out=alo[:, c * V2 : (c + 1) * V2],
                                            in0=lov[:],
                                            scalar1=lo_f[:, c : c + 1] if hasattr(lo_f, "__getitem__") else lo_f,
                                            scalar2=None, op0=Alu.is_equal)
                    bh = idxp.tile([128, PAIRS], mmdt, tag=f"bh{c}")
                    nc.vector.tensor_scalar(out=bh[:], in0=gidx[:],
                                            scalar1=hi_f[:, c : c + 1],
                                            scalar2=None, op0=Alu.is_equal)
                    bhs.append(bh)
                # gt split by pair-halves; emit all first halves before the
                # second halves so tiles 0-4 unblock as early as possible
                for p0, p1 in ((0, 16), (16, O)):
                    for c in range(NCHUNK):
                        at = at_all[:, c * O : (c + 1) * O]
                        nc.vector.tensor_tensor(
                            out=gt[:, c * PAIRS + p0 * V1 : c * PAIRS + p1 * V1]
                            .rearrange("p (o g) -> p o g", g=V1),
                            in0=at[:, p0:p1].unsqueeze(2).broadcast_to(
                                [128, p1 - p0, V1]),
                            in1=bhs[c][:, p0 * V1 : p1 * V1].rearrange(
                                "p (o g) -> p o g", g=V1),
                            op=Alu.mult,
                        )

                out_flat = out_d[b].rearrange("o v -> (o v)")
                for t in range(NTILES):
                    w = 128
                    ps = psmm.tile([128, V2], f32, tag="mm")
                    for c in range(NCHUNK):
                        for n0, n1 in NSLICES:
                            nc.tensor.matmul(
                                out=ps[:w, n0:n1],
                                lhsT=gt[:, c * PAIRS + t * 128 : c * PAIRS + t * 128 + w],
                                rhs=alo[:, c * V2 + n0 : c * V2 + n1],
                                start=(c == 0),
                                stop=(c == NCHUNK - 1),
                            )
                    os_ = outp.tile([128, V2], f32, tag="os")
                    last = b == BPC - 1 and t == NTILES - 1
                    halves = ((0, V2 // 2), (V2 // 2, V2)) if last else ((0, V2),)
                    for k, (v0, v1) in enumerate(halves):
                        nc.scalar.copy(out=os_[:w, v0:v1], in_=ps[:w, v0:v1])
                        nc.sync.dma_start(
                            out=out_flat[t * 128 * V2 : t * 128 * V2 + w * V2]
                            .rearrange("(p l) -> p l", l=V2)[:, v0:v1],
                            in_=os_[:w, v0:v1],
                        )

    nc.compile()
    return nc


def _consts():
    gidx = np.broadcast_to(
        np.tile(np.arange(V1, dtype=np.float16), O)[None, :], (128, PAIRS)
    ).copy()
    lov = np.broadcast_to(
        np.arange(V2, dtype=np.float16)[None, :], (128, V2)
    ).copy()
    return gidx, lov


def kernel(ids, attn):
    from concourse.bass_utils import run_bass_kernel_spmd

    ids = np.ascontiguousarray(ids, dtype=np.int32)
    attn = np.ascontiguousarray(attn, dtype=np.float32)

    if "nc" not in _cache:
        _cache["nc"] = _build()
    nc = _cache["nc"]

    gidx, lov = _consts()
    # [B, I] -> per core [128, BPC*NCHUNK] with [p, b*NCHUNK+c] = ids[b, c*128+p]
    ids_t = ids.reshape(B, NCHUNK, 128).transpose(0, 2, 1)  # [B, 128, NCHUNK]
    ids_t = ids_t.reshape(NCORES, BPC, 128, NCHUNK).transpose(0, 2, 1, 3).reshape(
        NCORES, 128, BPC * NCHUNK)
    attn_t = attn.transpose(0, 2, 1)  # [B, I, O]
    core_ids = list(range(NCORES))
    in_maps = [
        {
            "ids": np.ascontiguousarray(ids_t[c]),
            "attn": np.ascontiguousarray(attn_t[c * BPC : (c + 1) * BPC]),
            "gidx": gidx,
            "lov": lov,
        }
        for c in core_ids
    ]
    res = run_bass_kernel_spmd(nc, in_maps, core_ids)
    out = np.concatenate([res.results[c]["out"] for c in core_ids], axis=0)
    return out


# revision 15
# speedup vs baseline: 1.1642x; 1.1642x over previous
"""Trainium2 Bass kernel for batched weighted scatter-add (AttentionCopy).

Computes out[b, o, v] = sum_i attn[b, o, i] * (ids[b, i] == v)
for ids [16, 512] int32 in [0, 50000), attn [16, 32, 512] f32,
out [16, 32, 50000] f32.

Strategy: pure data parallel over the batch dim — 2 batches per core on 8
cores. Per batch the output is built *densely* on-device via a two-level
one-hot factorization of the vocab index (v = g*2000 + lo, 25 groups):

  out[o, g, lo] = sum_i (attnT[i, o] * (hi_i == g)) * (lo_i == lo)
                = (G_(o,g)).T @ Alo      (PE matmul, contraction over i)

The masked matrix G.T[i, (o, g)] and the one-hot Alo[i, lo] are built with
iota-constant compares on the vector engine. With o-major (o, g) pairs and
V2 = 1250 (40 * 1250 = 50000 exactly), each [128, 1250] f32 PSUM tile maps
to a *flat contiguous* 640 KB run of the row-major [32, 50000] output, so
every DRAM write is a full-partition coalesced DMA (all 16 SDMA engines),
and the 1280 pairs split into exactly 10 full 128-partition tiles.

hi = ids // 1250 uses the round-to-nearest int cast of
(ids + 0.5) * (1/1250) - 0.5, validated exhaustively on HW for [0, 50000).
"""

import sys

sys.path.insert(0, "/opt/trn_rl_repo")

import numpy as np

NCORES = 8
B, O, I = 16, 32, 512
SIZE = 50000
BPC = B // NCORES  # batches per core
V2 = 1250  # lo range (3 PSUM banks per tile)
V1 = 40  # number of vocab groups: 40 * 1250 = 50000 exactly
PAIRS = O * V1  # 1280 (o, g) pairs per batch, o-major
NTILES = PAIRS // 128  # exactly 10 full 128-pair tiles
NCHUNK = I // 128  # 4 contraction chunks
# matmul N-slices of V2, each within one 2 KiB PSUM bank
NSLICES = [(0, 512), (512, 1024), (1024, 1250)]

_cache = {}


def _build(mm_dtype="bfloat16"):
    import concourse.bacc as bacc
    import concourse.mybir as mybir
    import concourse.tile as tile

    f32 = mybir.dt.float32
    f16 = mybir.dt.float16
    mmdt = getattr(mybir.dt, mm_dtype)
    i32 = mybir.dt.int32
    Alu = mybir.AluOpType

    nc = bacc.Bacc("TRN2", target_bir_lowering=False, debug=False, num_devices=NCORES)

    # ids pre-tiled on host to [BPC, 128, NCHUNK] with [p, c] = ids[c*128 + p]
    ids_d = nc.dram_tensor("ids", [128, BPC * NCHUNK], i32, kind="ExternalInput").ap()
    attn_d = nc.dram_tensor("attn", [BPC, I, O], f32, kind="ExternalInput").ap()
    gidx_d = nc.dram_tensor("gidx", [128, PAIRS], f16, kind="ExternalInput").ap()
    lov_d = nc.dram_tensor("lov", [128, V2], f16, kind="ExternalInput").ap()
    out_d = nc.dram_tensor("out", [BPC, O, SIZE], f32, kind="ExternalOutput").ap()

    with tile.TileContext(nc) as tc:
        with (
            tc.tile_pool(name="const", bufs=1) as constp,
            tc.tile_pool(name="inp", bufs=2) as inp,
            tc.tile_pool(name="idx", bufs=2) as idxp,
            tc.tile_pool(name="gt", bufs=2) as gtp,
            tc.tile_pool(name="outs", bufs=4) as outp,
            tc.tile_pool(name="psmm", bufs=2, space="PSUM") as psmm,
        ):
            warm = constp.tile([128, 256], mmdt)
            nc.gpsimd.memset(warm[:], 0)
            wps = psmm.tile([128, 256], f32, tag="mm")
            for _ in range(52):
                nc.tensor.matmul(out=wps[:, :256], lhsT=warm[:, :128],
                                 rhs=warm[:, :256], start=True, stop=True)
            NBC = BPC * NCHUNK
            ids_all = idxp.tile([128, NBC], i32)
            nc.scalar.dma_start(out=ids_all[:], in_=ids_d[:])
            at_alls = []
            for b in range(BPC):
                at_alls.append(inp.tile([128, NCHUNK * O], f32, tag=f"attn{b}", name=f"at_all{b}"))
            lov = constp.tile([128, V2], f16)
            nc.sync.dma_start(out=lov[:], in_=lov_d[:])
            gidx = constp.tile([128, PAIRS], f16)
            nc.sync.dma_start(out=gidx[:], in_=gidx_d[:])
            for b in range(BPC):
                nc.sync.dma_start(
                    out=at_alls[b][:].rearrange("p (c o) -> p c o", o=O),
                    in_=attn_d[b].rearrange("(c p) o -> p c o", p=128),
                )

            # hi = ids // 1250 via RTN int cast of (ids+0.5)/1250 - 0.5
            # (exact for [0, 50000), verified on HW); lo = ids - 1250*hi
            # computed for both batches in one [128, 8] pass
            ids_f = idxp.tile([128, NBC], f32)
            nc.vector.tensor_copy(out=ids_f[:], in_=ids_all[:])
            tq = idxp.tile([128, NBC], f32)
            nc.vector.tensor_scalar(out=tq[:], in0=ids_f[:], scalar1=0.5,
                                    scalar2=float(np.float32(1.0 / V2)),
                                    op0=Alu.add, op1=Alu.mult)
            hi_i = idxp.tile([128, NBC], i32)
            nc.vector.tensor_scalar(out=hi_i[:], in0=tq[:], scalar1=0.5,
                                    scalar2=None, op0=Alu.subtract)
            hi_fa = idxp.tile([128, NBC], f32)
            nc.vector.tensor_copy(out=hi_fa[:], in_=hi_i[:])
            lo_fa = idxp.tile([128, NBC], f32)
            nc.vector.scalar_tensor_tensor(out=lo_fa[:], in0=hi_fa[:],
                                           scalar=float(-V2), in1=ids_f[:],
                                           op0=Alu.mult, op1=Alu.add)

            for b in range(BPC):
                at_all = at_alls[b]
                hi_f = hi_fa[:, b * NCHUNK : (b + 1) * NCHUNK]
                lo_f = lo_fa[:, b * NCHUNK : (b + 1) * NCHUNK]

                gt = gtp.tile([128, NCHUNK * PAIRS], mmdt, tag="gt")
                alo = gtp.tile([128, NCHUNK * V2], mmdt, tag="alo")
                bhs = []
                for c in range(NCHUNK):
                    nc.vector.tensor_scalar(out=alo[:, c * V2 : (c + 1) * V2],
                                            in0=lov[:],
                                            scalar1=lo_f[:, c : c + 1] if hasattr(lo_f, "__getitem__") else lo_f,
                                            scalar2=None, op0=Alu.is_equal)
                    bh = idxp.tile([128, PAIRS], mmdt, tag=f"bh{c}")
                    nc.vector.tensor_scalar(out=bh[:], in0=gidx[:],
                                            scalar1=hi_f[:, c : c + 1],
                                            scalar2=None, op0=Alu.is_equal)
                    bhs.append(bh)
                # gt split by pair-halves; emit all first halves before the
                # second halves so tiles 0-4 unblock as early as possible
                for p0, p1 in ((0, 16), (16, O)):
                    for c in range(NCHUNK):
                        at = at_all[:, c * O : (c + 1) * O]
                        nc.vector.tensor_tensor(
                            out=gt[:, c * PAIRS + p0 * V1 : c * PAIRS + p1 * V1]
                            .rearrange("p (o g) -> p o g", g=V1),
                            in0=at[:, p0:p1].unsqueeze(2).broadcast_to(
                                [128, p1 - p0, V1]),
                            in1=bhs[c][:, p0 * V1 : p1 * V1].rearrange(
                                "p (o g) -> p o g", g=V1),
                            op=Alu.mult,
                        )

                out_flat = out_d[b].rearrange("o v -> (o v)")
                for t in range(NTILES):
                    w = 128
                    ps = psmm.tile([128, V2], f32, tag="mm")
                    for c in range(NCHUNK):
                        for n0, n1 in NSLICES:
                            nc.tensor.matmul(
                                out=ps[:w, n0:n1],
                                lhsT=gt[:, c * PAIRS + t * 128 : c * PAIRS + t * 128 + w],
                                rhs=alo[:, c * V2 + n0 : c * V2 + n1],
                                start=(c == 0),
                                stop=(c == NCHUNK - 1),
                            )
                    os_ = outp.tile([128, V2], f32, tag="os")
                    last = b == BPC - 1 and t == NTILES - 1
                    halves = ((0, V2 // 2), (V2 // 2, V2)) if last else ((0, V2),)
                    for k, (v0, v1) in enumerate(halves):
                        nc.scalar.copy(out=os_[:w, v0:v1], in_=ps[:w, v0:v1])
                        nc.sync.dma_start(
                            out=out_flat[t * 128 * V2 : t * 128 * V2 + w * V2]
                            .rearrange("(p l) -> p l", l=V2)[:, v0:v1],
                            in_=os_[:w, v0:v1],
                        )

    nc.compile()
    return nc


def _consts():
    gidx = np.broadcast_to(
        np.tile(np.arange(V1, dtype=np.float16), O)[None, :], (128, PAIRS)
    ).copy()
    lov = np.broadcast_to(
        np.arange(V2, dtype=np.float16)[None, :], (128, V2)
    ).copy()
    return gidx, lov


def _in_maps(ids, attn):
    gidx, lov = _consts()
    # [B, I] -> per core [128, BPC*NCHUNK] with [p, b*NCHUNK+c] = ids[b, c*128+p]
    ids_t = ids.reshape(B, NCHUNK, 128).transpose(0, 2, 1)  # [B, 128, NCHUNK]
    ids_t = ids_t.reshape(NCORES, BPC, 128, NCHUNK).transpose(0, 2, 1, 3).reshape(
        NCORES, 128, BPC * NCHUNK)
    attn_t = attn.transpose(0, 2, 1)  # [B, I, O]
    in_maps = [
        {
            "ids": np.ascontiguousarray(ids_t[c]),
            "attn": np.ascontiguousarray(attn_t[c * BPC : (c + 1) * BPC]),
            "gidx": gidx,
            "lov": lov,
        }
        for c in range(NCORES)
    ]
    return in_maps


def kernel(ids, attn):
    from concourse.bass_utils import run_bass_kernel_spmd

    ids = np.ascontiguousarray(ids, dtype=np.int32)
    attn = np.ascontiguousarray(attn, dtype=np.float32)

    if "nc" not in _cache:
        _cache["nc"] = _build()
    nc = _cache["nc"]

    core_ids = list(range(NCORES))
    res = run_bass_kernel_spmd(nc, _in_maps(ids, attn), core_ids)
    out = np.concatenate([res.results[c]["out"] for c in core_ids], axis=0)
    return out
